# revision 55
# baseline (speedup 1.0000x reference)
"""Trainium2 Bass kernel for BilinearInteraction.

out[b, p] = x[b, i_p, :] @ W[p] @ x[b, j_p, :]  for the 780 field pairs
(i, j), i < j, of F=40 fields (row-major triu order).

Architecture (8 NeuronCores, data-parallel over batch, B_loc=256):
  - "b-T" layout: stage-1 PE matmuls produce Y[(pair, e), b] in PSUM
    (pairs x e on partitions, batch on the free dim), so the final
    e-reduction runs on the PE as ones-mask matmuls (contraction over
    partitions), keeping the vector engine to a single multiply pass.
  - Tiles: one [128, 256] PSUM slice holds 2 pairs sharing one i-field:
    (i, 2t) and (i, 2t+1), matching xT chunk t (fields 2t / 2t+1 on the
    two partition halves). W is host-permuted (bf16) into per-tile
    contiguous lhsT blocks (zero blocks for invalid (i==2t, 2t) slots).
    4 tiles of one chunk share a 2-bank [128, 1024] PSUM group so one
    tensor_tensor covers 4 tiles (amortizes the DVE PSUM-access bubble).
  - Host pre-transposes x into the three layouts the kernel needs
    (xtc f32 / xtcb bf16 for the multiply, xtlo bf16 for stage-1 rhs),
    eliminating all on-device transposes.
  - stage 1: PE matmul Y = Wtile.T @ xT_i  (bf16, K=64, M=128, N=256).
  - stage 2: ACT evicts Y -> bf16 SBUF; DVE multiplies by xtcb chunk at
    the 2x packed rate -> z bf16 (leftover small groups multiply PSUM
    directly on DVE at 1x).
  - stage 3: PE ones-mask matmuls (K=128, M=32) accumulate 16 z-tiles
    into one PSUM bank = 32 output pair-rows (full fp32 accumulation).
    Col-group tiling is deliberately NOT used: tile_position col-groups
    interleaved with full-width matmuls corrupt nondeterministically on
    real TRN2 hardware (verified; CoreSim is clean).
  - ACT evicts each result bank; DMA to outT[bank*32 + row, b]; the
    host inverse-permutes pair rows and concatenates the batch shards.
"""

import numpy as np
import ml_dtypes

import concourse.bass as bass
import concourse.mybir as mybir
import concourse.tile as tile
from concourse import bacc
from concourse.bass_utils import run_bass_kernel_spmd

B, F, D = 2048, 40, 64
P = F * (F - 1) // 2  # 780
NCORES = 8
B_LOC = B // NCORES  # 256
F32 = mybir.dt.float32
BF16 = mybir.dt.bfloat16

NCHUNK = F // 2  # 20 xT chunks (2 fields each)
# tile list: (t, i) — pairs (i, 2t) [dummy if i==2t] and (i, 2t+1)
TILES = [(t, i) for t in range(NCHUNK) for i in range(2 * t + 1)]
NTILES = len(TILES)  # 400
REDUCE_COLTILE = False
TILES_PER_BANK = 64 if REDUCE_COLTILE else 16
BANK_ROWS = 128 if REDUCE_COLTILE else 32
NBANKS = (NTILES + TILES_PER_BANK - 1) // TILES_PER_BANK
OUT_ROWS = NBANKS * BANK_ROWS


GMAX = 4  # tiles per PSUM group (4 x 256 cols = 2 banks)
REDUCE_DELAY = 5  # groups of reduce-matmul lag (software pipelining)


def _build_groups():
    # pairs of same-chunk tiles sharing one [128,512] PSUM bank; split at
    # bank boundaries so both reduce slots land in the same bank-pass
    groups = []
    k = 0
    for t in range(NCHUNK):
        ilist = list(range(2 * t + 1))
        while ilist:
            take = min(GMAX, len(ilist), TILES_PER_BANK - (k % TILES_PER_BANK))
            groups.append((t, ilist[:take]))
            ilist = ilist[take:]
            k += take
    return groups


GROUPS = _build_groups()

WDMA_BATCH = 8  # stage-1 lhsT tiles per DMA


def host_prep(W: np.ndarray):
    """Build Wt3 [64, NTILES*128] f32, ONES [128, 512] bf16, PERM info."""
    # Wt2[d, p, e]
    Wt2 = np.ascontiguousarray(W.transpose(1, 0, 2))  # [64, 780, 64]
    pair_idx = -np.ones((F, F), dtype=np.int64)
    k = 0
    for i in range(F):
        for j in range(i + 1, F):
            pair_idx[i, j] = k
            k += 1
    Wt3 = np.zeros((D, NTILES * 128), dtype=np.float32)  # cast to bf16 at end
    # rows[k] = (origA or -1, origB) for tile k
    rows = []
    for k, (t, i) in enumerate(TILES):
        jA, jB = 2 * t, 2 * t + 1
        pA = pair_idx[i, jA] if i < jA else -1
        pB = pair_idx[i, jB]
        if pA >= 0:
            Wt3[:, k * 128 : k * 128 + 64] = Wt2[:, pA, :]
        Wt3[:, k * 128 + 64 : k * 128 + 128] = Wt2[:, pB, :]
        rows.append((pA, pB))
    # ones masks: ONES[:, q*32+m] — slot q (0..15): col 2q active for k<64,
    # col 2q+1 active for k>=64
    ones = np.zeros((128, 512), dtype=np.float32)
    for q in range(16):
        ones[0:64, q * 32 + 2 * q] = 1.0
        ones[64:128, q * 32 + 2 * q + 1] = 1.0
    ones = ones.astype(ml_dtypes.bfloat16)
    # out row of tile k: bank = k//64, s = k%64, g = s%4, q = s//4
    # rowA = bank*128 + 32*g + 2*q ; rowB = rowA + 1
    perm_src = np.zeros(P, dtype=np.int64)  # outT row for original pair p
    for k, (pA, pB) in enumerate(rows):
        bank, s = divmod(k, TILES_PER_BANK)
        if REDUCE_COLTILE:
            g, q = s % 4, s // 4
            rowA = bank * BANK_ROWS + 32 * g + 2 * q
        else:
            rowA = bank * BANK_ROWS + 2 * s
        if pA >= 0:
            perm_src[pA] = rowA
        perm_src[pB] = rowA + 1
    return Wt3.astype(ml_dtypes.bfloat16), ones, perm_src


def build_nc():
    nc = bacc.Bacc("TRN2", target_bir_lowering=False, debug=False)

    xtc_dram = nc.dram_tensor(
        "xtc", [128, NCHUNK * B_LOC], F32, kind="ExternalInput"
    ).ap()
    xtlo_dram = nc.dram_tensor(
        "xtlo", [64, F * B_LOC], BF16, kind="ExternalInput"
    ).ap()
    xtcb_dram = nc.dram_tensor(
        "xtcb", [128, NCHUNK * B_LOC], BF16, kind="ExternalInput"
    ).ap()
    wt_dram = nc.dram_tensor("Wt3", [D, NTILES * 128], BF16, kind="ExternalInput").ap()
    ones_dram = nc.dram_tensor("ones", [128, 512], BF16, kind="ExternalInput").ap()
    out_dram = nc.dram_tensor("outT", [OUT_ROWS, B_LOC], F32, kind="ExternalOutput").ap()

    with tile.TileContext(nc) as tc:
        with (
            tc.tile_pool(name="persist", bufs=1) as persist,
            tc.tile_pool(name="wpool", bufs=4) as wpool,
            tc.tile_pool(name="zpool", bufs=12) as zpool,
            tc.tile_pool(name="ybfpool", bufs=6) as ybfpool,
            tc.tile_pool(name="opool", bufs=2) as opool,
            tc.tile_pool(name="ypsum", bufs=3, space=bass.MemorySpace.PSUM) as ypsum,
            tc.tile_pool(name="rpsum", bufs=2, space=bass.MemorySpace.PSUM) as rpsum,
        ):
            ones = persist.tile([128, 512], BF16, tag="ones")
            nc.sync.dma_start(out=ones[:], in_=ones_dram[:])

            # XTC[(f%2)*64 + d, t*256 + m*128 + b]  (f = 2t + f%2) and the
            # low-half layout (all fields at partitions 0-63) are both
            # pre-transposed on the host and DMA'd directly.
            xtc = persist.tile([128, NCHUNK * B_LOC], F32, tag="xtc")
            xtlo = persist.tile([64, F * B_LOC], BF16, tag="xtlo")
            xtcb = persist.tile([128, NCHUNK * B_LOC], BF16, tag="xtcb")
            nq = NCHUNK * B_LOC // 4
            nf = F * B_LOC // 4
            for c4 in range(4):
                nc.sync.dma_start(
                    out=xtc[:, c4 * nq : (c4 + 1) * nq],
                    in_=xtc_dram[:, c4 * nq : (c4 + 1) * nq],
                )
                nc.sync.dma_start(
                    out=xtlo[:, c4 * nf : (c4 + 1) * nf],
                    in_=xtlo_dram[:, c4 * nf : (c4 + 1) * nf],
                )
                nc.sync.dma_start(
                    out=xtcb[:, c4 * nq : (c4 + 1) * nq],
                    in_=xtcb_dram[:, c4 * nq : (c4 + 1) * nq],
                )

            rbs = [None]
            wchunk = None
            k = 0
            nquad = 0
            pending = []

            def emit_reduce(z, k0, gsz):
                # accumulate into reduce bank via ones-mask matmuls
                for idx in range(gsz):
                    kt = k0 + idx
                    bank, s = divmod(kt, TILES_PER_BANK)
                    q = s
                    if s == 0:
                        rbs[0] = rpsum.tile([128, B_LOC], F32, tag="rb", name="rb")
                    rb = rbs[0]
                    last_in_bank = (s == TILES_PER_BANK - 1) or (kt == NTILES - 1)
                    nc.tensor.matmul(
                        rb[0:32, :],
                        ones[:, q * 32 : (q + 1) * 32],
                        z[:, idx * B_LOC : (idx + 1) * B_LOC],
                        start=(s == 0),
                        stop=last_in_bank,
                        tile_position=(0, 0),
                        skip_group_check=True,
                    )
                    if last_in_bank:
                        ob = opool.tile([BANK_ROWS, B_LOC], F32, tag="ob")
                        nc.vector.tensor_copy(out=ob[:], in_=rb[0:BANK_ROWS, :])
                        nc.sync.dma_start(
                            out=out_dram[
                                bank * BANK_ROWS : (bank + 1) * BANK_ROWS, :
                            ],
                            in_=ob[:],
                        )

            for t, ilist in GROUPS:
                gsz = len(ilist)
                # stage 1: Y[(p, e), b] = Wtile.T @ xT_i — gsz tiles share
                # one PSUM bank (disjoint column halves)
                y = ypsum.tile([128, GMAX * B_LOC], F32, tag="y")
                for idx, i in enumerate(ilist):
                    kt = k + idx
                    if kt % WDMA_BATCH == 0:
                        nw = min(WDMA_BATCH, NTILES - kt)
                        wchunk = wpool.tile([64, WDMA_BATCH * 128], BF16, tag="w")
                        nc.sync.dma_start(
                            out=wchunk[:, : nw * 128],
                            in_=wt_dram[:, kt * 128 : (kt + nw) * 128],
                        )
                    kk = kt % WDMA_BATCH
                    nc.tensor.matmul(
                        y[:, idx * B_LOC : (idx + 1) * B_LOC],
                        wchunk[:, kk * 128 : (kk + 1) * 128],
                        xtlo[:, i * B_LOC : (i + 1) * B_LOC],
                        start=True,
                        stop=True,
                    )

                # stage 2: z = Y * xT[j-fields chunk t]  (bf16 out), one TT
                # per group with stride-0 broadcast of the xtc chunk.
                # A fraction of pair-groups takes the ACT-evict + GPSIMD
                # multiply path to unload the DVE.
                z = zpool.tile([128, GMAX * B_LOC], BF16, tag="z")
                if gsz > 1:
                    nquad += 1
                    act_path = gsz == GMAX and (nquad % 4) < 2
                    if gsz == GMAX:
                        nquad += 1
                    if act_path:
                        # ACT evicts PSUM -> bf16, DVE multiplies at 2x
                        ybf = ybfpool.tile([128, GMAX * B_LOC], BF16, tag="ybf")
                        if nquad % 9 == 1:
                            # spill ~1/18 of the PSUM drains to the DVE to
                            # equalize ACT/DVE/PE busy time
                            nc.vector.tensor_copy(out=ybf[:], in_=y[:])
                        else:
                            nc.scalar.copy(out=ybf[:], in_=y[:])
                        in1 = xtcb[
                            :, None, t * B_LOC : (t + 1) * B_LOC
                        ].to_broadcast([128, gsz, B_LOC])
                        nc.vector.tensor_tensor(
                            z[:, : gsz * B_LOC].rearrange(
                                "p (n b) -> p n b", n=gsz
                            ),
                            ybf[:, : gsz * B_LOC].rearrange(
                                "p (n b) -> p n b", n=gsz
                            ),
                            in1,
                            mybir.AluOpType.mult,
                        )
                    else:
                        in1 = xtc[
                            :, None, t * B_LOC : (t + 1) * B_LOC
                        ].to_broadcast([128, gsz, B_LOC])
                        nc.vector.tensor_tensor(
                            z[:, : gsz * B_LOC].rearrange(
                                "p (n b) -> p n b", n=gsz
                            ),
                            y[:, : gsz * B_LOC].rearrange(
                                "p (n b) -> p n b", n=gsz
                            ),
                            in1,
                            mybir.AluOpType.mult,
                        )
                else:
                    nc.vector.tensor_tensor(
                        z[:, :B_LOC],
                        y[:, :B_LOC],
                        xtc[:, t * B_LOC : (t + 1) * B_LOC],
                        mybir.AluOpType.mult,
                    )

                # stage 3 is software-pipelined: queue this group's reduce
                # and emit the one from REDUCE_DELAY groups ago, so the PE's
                # in-order queue never waits on the just-issued ACT->DVE
                # multiply chain.
                pending.append((z, k, gsz))
                if len(pending) > REDUCE_DELAY:
                    emit_reduce(*pending.pop(0))
                k += gsz

            while pending:
                emit_reduce(*pending.pop(0))

    nc.compile()
    return nc


_NC = None


def kernel(x: np.ndarray, W: np.ndarray) -> np.ndarray:
    global _NC
    x = np.ascontiguousarray(np.asarray(x, dtype=np.float32))
    W = np.ascontiguousarray(np.asarray(W, dtype=np.float32))
    assert x.shape == (B, F, D) and W.shape == (P, D, D)

    Wt3, ones, perm_src = host_prep(W)

    if _NC is None:
        _NC = build_nc()

    in_maps = []
    for c in range(NCORES):
        xs = x[c * B_LOC : (c + 1) * B_LOC]  # [256, 40, 64]
        v = xs.transpose(1, 2, 0).reshape(NCHUNK, 2, D, B_LOC)
        xtc = np.ascontiguousarray(
            v.transpose(1, 2, 0, 3).reshape(128, NCHUNK * B_LOC)
        )
        xtlo = np.ascontiguousarray(
            xs.transpose(2, 1, 0).reshape(D, F * B_LOC)
        ).astype(ml_dtypes.bfloat16)
        xtcb = xtc.astype(ml_dtypes.bfloat16)
        in_maps.append(
            {"xtc": xtc, "xtcb": xtcb, "xtlo": xtlo, "Wt3": Wt3, "ones": ones}
        )
    res = run_bass_kernel_spmd(_NC, in_maps, core_ids=list(range(NCORES)))
    out = np.empty((B, P), dtype=np.float32)
    for c in range(NCORES):
        outT = res.results[c]["outT"]  # [OUT_ROWS, B_LOC]
        out[c * B_LOC : (c + 1) * B_LOC, :] = outT[perm_src, :].T
    return out


# revision 58
# speedup vs baseline: 1.0028x; 1.0028x over previous
"""Trainium2 Bass kernel for BilinearInteraction.

out[b, p] = x[b, i_p, :] @ W[p] @ x[b, j_p, :]  for the 780 field pairs
(i, j), i < j, of F=40 fields (row-major triu order).

Architecture (8 NeuronCores, data-parallel over batch, B_loc=256):
  - "b-T" layout: stage-1 PE matmuls produce Y[(pair, e), b] in PSUM
    (pairs x e on partitions, batch on the free dim), so the final
    e-reduction runs on the PE as ones-mask matmuls (contraction over
    partitions), keeping the vector engine to a single multiply pass.
  - Tiles: one [128, 256] PSUM slice holds 2 pairs sharing one i-field:
    (i, 2t) and (i, 2t+1), matching xT chunk t (fields 2t / 2t+1 on the
    two partition halves). W is host-permuted (bf16) into per-tile
    contiguous lhsT blocks (zero blocks for invalid (i==2t, 2t) slots).
    4 tiles of one chunk share a 2-bank [128, 1024] PSUM group so one
    tensor_tensor covers 4 tiles (amortizes the DVE PSUM-access bubble).
  - Host pre-transposes x into the three layouts the kernel needs
    (xtc f32 / xtcb bf16 for the multiply, xtlo bf16 for stage-1 rhs),
    eliminating all on-device transposes.
  - stage 1: PE matmul Y = Wtile.T @ xT_i  (bf16, K=64, M=128, N=256).
  - stage 2: ACT evicts Y -> bf16 SBUF; DVE multiplies by xtcb chunk at
    the 2x packed rate -> z bf16 (leftover small groups multiply PSUM
    directly on DVE at 1x).
  - stage 3: PE ones-mask matmuls (K=128, M=32) accumulate 16 z-tiles
    into one PSUM bank = 32 output pair-rows (full fp32 accumulation).
    Col-group tiling is deliberately NOT used: tile_position col-groups
    interleaved with full-width matmuls corrupt nondeterministically on
    real TRN2 hardware (verified; CoreSim is clean).
  - ACT evicts each result bank; DMA to outT[bank*32 + row, b]; the
    host inverse-permutes pair rows and concatenates the batch shards.
"""

import numpy as np
import ml_dtypes

import concourse.bass as bass
import concourse.mybir as mybir
import concourse.tile as tile
from concourse import bacc
from concourse.bass_utils import run_bass_kernel_spmd

B, F, D = 2048, 40, 64
P = F * (F - 1) // 2  # 780
NCORES = 8
B_LOC = B // NCORES  # 256
F32 = mybir.dt.float32
BF16 = mybir.dt.bfloat16

NCHUNK = F // 2  # 20 xT chunks (2 fields each)
# tile list: (t, i) — pairs (i, 2t) [dummy if i==2t] and (i, 2t+1)
TILES = [(t, i) for t in range(NCHUNK) for i in range(2 * t + 1)]
NTILES = len(TILES)  # 400
REDUCE_COLTILE = False
TILES_PER_BANK = 64 if REDUCE_COLTILE else 16
BANK_ROWS = 128 if REDUCE_COLTILE else 32
NBANKS = (NTILES + TILES_PER_BANK - 1) // TILES_PER_BANK
OUT_ROWS = NBANKS * BANK_ROWS


GMAX = 4  # tiles per PSUM group (4 x 256 cols = 2 banks)
REDUCE_DELAY = 7  # groups of reduce-matmul lag (software pipelining)


def _build_groups():
    # pairs of same-chunk tiles sharing one [128,512] PSUM bank; split at
    # bank boundaries so both reduce slots land in the same bank-pass
    groups = []
    k = 0
    for t in range(NCHUNK):
        ilist = list(range(2 * t + 1))
        while ilist:
            take = min(GMAX, len(ilist), TILES_PER_BANK - (k % TILES_PER_BANK))
            groups.append((t, ilist[:take]))
            ilist = ilist[take:]
            k += take
    return groups


GROUPS = _build_groups()

WDMA_BATCH = 8  # stage-1 lhsT tiles per DMA


def host_prep(W: np.ndarray):
    """Build Wt3 [64, NTILES*128] f32, ONES [128, 512] bf16, PERM info."""
    # Wt2[d, p, e]
    Wt2 = np.ascontiguousarray(W.transpose(1, 0, 2))  # [64, 780, 64]
    pair_idx = -np.ones((F, F), dtype=np.int64)
    k = 0
    for i in range(F):
        for j in range(i + 1, F):
            pair_idx[i, j] = k
            k += 1
    Wt3 = np.zeros((D, NTILES * 128), dtype=np.float32)  # cast to bf16 at end
    # rows[k] = (origA or -1, origB) for tile k
    rows = []
    for k, (t, i) in enumerate(TILES):
        jA, jB = 2 * t, 2 * t + 1
        pA = pair_idx[i, jA] if i < jA else -1
        pB = pair_idx[i, jB]
        if pA >= 0:
            Wt3[:, k * 128 : k * 128 + 64] = Wt2[:, pA, :]
        Wt3[:, k * 128 + 64 : k * 128 + 128] = Wt2[:, pB, :]
        rows.append((pA, pB))
    # ones masks: ONES[:, q*32+m] — slot q (0..15): col 2q active for k<64,
    # col 2q+1 active for k>=64
    ones = np.zeros((128, 512), dtype=np.float32)
    for q in range(16):
        ones[0:64, q * 32 + 2 * q] = 1.0
        ones[64:128, q * 32 + 2 * q + 1] = 1.0
    ones = ones.astype(ml_dtypes.bfloat16)
    # out row of tile k: bank = k//64, s = k%64, g = s%4, q = s//4
    # rowA = bank*128 + 32*g + 2*q ; rowB = rowA + 1
    perm_src = np.zeros(P, dtype=np.int64)  # outT row for original pair p
    for k, (pA, pB) in enumerate(rows):
        bank, s = divmod(k, TILES_PER_BANK)
        if REDUCE_COLTILE:
            g, q = s % 4, s // 4
            rowA = bank * BANK_ROWS + 32 * g + 2 * q
        else:
            rowA = bank * BANK_ROWS + 2 * s
        if pA >= 0:
            perm_src[pA] = rowA
        perm_src[pB] = rowA + 1
    return Wt3.astype(ml_dtypes.bfloat16), ones, perm_src


def build_nc():
    nc = bacc.Bacc("TRN2", target_bir_lowering=False, debug=False)

    xtc_dram = nc.dram_tensor(
        "xtc", [128, NCHUNK * B_LOC], F32, kind="ExternalInput"
    ).ap()
    xtlo_dram = nc.dram_tensor(
        "xtlo", [64, F * B_LOC], BF16, kind="ExternalInput"
    ).ap()
    xtcb_dram = nc.dram_tensor(
        "xtcb", [128, NCHUNK * B_LOC], BF16, kind="ExternalInput"
    ).ap()
    wt_dram = nc.dram_tensor("Wt3", [D, NTILES * 128], BF16, kind="ExternalInput").ap()
    ones_dram = nc.dram_tensor("ones", [128, 512], BF16, kind="ExternalInput").ap()
    out_dram = nc.dram_tensor("outT", [OUT_ROWS, B_LOC], F32, kind="ExternalOutput").ap()

    with tile.TileContext(nc) as tc:
        with (
            tc.tile_pool(name="persist", bufs=1) as persist,
            tc.tile_pool(name="wpool", bufs=4) as wpool,
            tc.tile_pool(name="zpool", bufs=12) as zpool,
            tc.tile_pool(name="ybfpool", bufs=6) as ybfpool,
            tc.tile_pool(name="opool", bufs=2) as opool,
            tc.tile_pool(name="ypsum", bufs=3, space=bass.MemorySpace.PSUM) as ypsum,
            tc.tile_pool(name="rpsum", bufs=2, space=bass.MemorySpace.PSUM) as rpsum,
        ):
            ones = persist.tile([128, 512], BF16, tag="ones")
            nc.sync.dma_start(out=ones[:], in_=ones_dram[:])

            # XTC[(f%2)*64 + d, t*256 + m*128 + b]  (f = 2t + f%2) and the
            # low-half layout (all fields at partitions 0-63) are both
            # pre-transposed on the host and DMA'd directly.
            xtc = persist.tile([128, NCHUNK * B_LOC], F32, tag="xtc")
            xtlo = persist.tile([64, F * B_LOC], BF16, tag="xtlo")
            xtcb = persist.tile([128, NCHUNK * B_LOC], BF16, tag="xtcb")
            nq = NCHUNK * B_LOC // 4
            nf = F * B_LOC // 4
            for c4 in range(4):
                nc.sync.dma_start(
                    out=xtc[:, c4 * nq : (c4 + 1) * nq],
                    in_=xtc_dram[:, c4 * nq : (c4 + 1) * nq],
                )
                nc.sync.dma_start(
                    out=xtlo[:, c4 * nf : (c4 + 1) * nf],
                    in_=xtlo_dram[:, c4 * nf : (c4 + 1) * nf],
                )
                nc.sync.dma_start(
                    out=xtcb[:, c4 * nq : (c4 + 1) * nq],
                    in_=xtcb_dram[:, c4 * nq : (c4 + 1) * nq],
                )

            rbs = [None]
            wchunk = None
            k = 0
            nquad = 0
            pending = []

            def emit_reduce(z, k0, gsz):
                # accumulate into reduce bank via ones-mask matmuls
                for idx in range(gsz):
                    kt = k0 + idx
                    bank, s = divmod(kt, TILES_PER_BANK)
                    q = s
                    if s == 0:
                        rbs[0] = rpsum.tile([128, B_LOC], F32, tag="rb", name="rb")
                    rb = rbs[0]
                    last_in_bank = (s == TILES_PER_BANK - 1) or (kt == NTILES - 1)
                    nc.tensor.matmul(
                        rb[0:32, :],
                        ones[:, q * 32 : (q + 1) * 32],
                        z[:, idx * B_LOC : (idx + 1) * B_LOC],
                        start=(s == 0),
                        stop=last_in_bank,
                        tile_position=(0, 0),
                        skip_group_check=True,
                    )
                    if last_in_bank:
                        ob = opool.tile([BANK_ROWS, B_LOC], F32, tag="ob")
                        nc.vector.tensor_copy(out=ob[:], in_=rb[0:BANK_ROWS, :])
                        nc.sync.dma_start(
                            out=out_dram[
                                bank * BANK_ROWS : (bank + 1) * BANK_ROWS, :
                            ],
                            in_=ob[:],
                        )

            for t, ilist in GROUPS:
                gsz = len(ilist)
                # stage 1: Y[(p, e), b] = Wtile.T @ xT_i — gsz tiles share
                # one PSUM bank (disjoint column halves)
                y = ypsum.tile([128, GMAX * B_LOC], F32, tag="y")
                for idx, i in enumerate(ilist):
                    kt = k + idx
                    if kt % WDMA_BATCH == 0:
                        nw = min(WDMA_BATCH, NTILES - kt)
                        wchunk = wpool.tile([64, WDMA_BATCH * 128], BF16, tag="w")
                        nc.sync.dma_start(
                            out=wchunk[:, : nw * 128],
                            in_=wt_dram[:, kt * 128 : (kt + nw) * 128],
                        )
                    kk = kt % WDMA_BATCH
                    nc.tensor.matmul(
                        y[:, idx * B_LOC : (idx + 1) * B_LOC],
                        wchunk[:, kk * 128 : (kk + 1) * 128],
                        xtlo[:, i * B_LOC : (i + 1) * B_LOC],
                        start=True,
                        stop=True,
                    )

                # stage 2: z = Y * xT[j-fields chunk t]  (bf16 out), one TT
                # per group with stride-0 broadcast of the xtc chunk.
                # A fraction of pair-groups takes the ACT-evict + GPSIMD
                # multiply path to unload the DVE.
                z = zpool.tile([128, GMAX * B_LOC], BF16, tag="z")
                if gsz > 1:
                    nquad += 1
                    act_path = gsz == GMAX and (nquad % 4) < 2
                    if gsz == GMAX:
                        nquad += 1
                    if act_path:
                        # ACT evicts PSUM -> bf16, DVE multiplies at 2x
                        ybf = ybfpool.tile([128, GMAX * B_LOC], BF16, tag="ybf")
                        if nquad % 9 == 1:
                            # spill ~1/18 of the PSUM drains to the DVE to
                            # equalize ACT/DVE/PE busy time
                            nc.vector.tensor_copy(out=ybf[:], in_=y[:])
                        else:
                            nc.scalar.copy(out=ybf[:], in_=y[:])
                        in1 = xtcb[
                            :, None, t * B_LOC : (t + 1) * B_LOC
                        ].to_broadcast([128, gsz, B_LOC])
                        nc.vector.tensor_tensor(
                            z[:, : gsz * B_LOC].rearrange(
                                "p (n b) -> p n b", n=gsz
                            ),
                            ybf[:, : gsz * B_LOC].rearrange(
                                "p (n b) -> p n b", n=gsz
                            ),
                            in1,
                            mybir.AluOpType.mult,
                        )
                    else:
                        in1 = xtc[
                            :, None, t * B_LOC : (t + 1) * B_LOC
                        ].to_broadcast([128, gsz, B_LOC])
                        nc.vector.tensor_tensor(
                            z[:, : gsz * B_LOC].rearrange(
                                "p (n b) -> p n b", n=gsz
                            ),
                            y[:, : gsz * B_LOC].rearrange(
                                "p (n b) -> p n b", n=gsz
                            ),
                            in1,
                            mybir.AluOpType.mult,
                        )
                else:
                    nc.vector.tensor_tensor(
                        z[:, :B_LOC],
                        y[:, :B_LOC],
                        xtc[:, t * B_LOC : (t + 1) * B_LOC],
                        mybir.AluOpType.mult,
                    )

                # stage 3 is software-pipelined: queue this group's reduce
                # and emit the one from REDUCE_DELAY groups ago, so the PE's
                # in-order queue never waits on the just-issued ACT->DVE
                # multiply chain.
                pending.append((z, k, gsz))
                if len(pending) > REDUCE_DELAY:
                    emit_reduce(*pending.pop(0))
                k += gsz

            while pending:
                emit_reduce(*pending.pop(0))

    nc.compile()
    return nc


_NC = None


def kernel(x: np.ndarray, W: np.ndarray) -> np.ndarray:
    global _NC
    x = np.ascontiguousarray(np.asarray(x, dtype=np.float32))
    W = np.ascontiguousarray(np.asarray(W, dtype=np.float32))
    assert x.shape == (B, F, D) and W.shape == (P, D, D)

    Wt3, ones, perm_src = host_prep(W)

    if _NC is None:
        _NC = build_nc()

    in_maps = []
    for c in range(NCORES):
        xs = x[c * B_LOC : (c + 1) * B_LOC]  # [256, 40, 64]
        v = xs.transpose(1, 2, 0).reshape(NCHUNK, 2, D, B_LOC)
        xtc = np.ascontiguousarray(
            v.transpose(1, 2, 0, 3).reshape(128, NCHUNK * B_LOC)
        )
        xtlo = np.ascontiguousarray(
            xs.transpose(2, 1, 0).reshape(D, F * B_LOC)
        ).astype(ml_dtypes.bfloat16)
        xtcb = xtc.astype(ml_dtypes.bfloat16)
        in_maps.append(
            {"xtc": xtc, "xtcb": xtcb, "xtlo": xtlo, "Wt3": Wt3, "ones": ones}
        )
    res = run_bass_kernel_spmd(_NC, in_maps, core_ids=list(range(NCORES)))
    out = np.empty((B, P), dtype=np.float32)
    for c in range(NCORES):
        outT = res.results[c]["outT"]  # [OUT_ROWS, B_LOC]
        out[c * B_LOC : (c + 1) * B_LOC, :] = outT[perm_src, :].T
    return out


# revision 59
# speedup vs baseline: 1.0107x; 1.0078x over previous
"""Trainium2 Bass kernel for BilinearInteraction.

out[b, p] = x[b, i_p, :] @ W[p] @ x[b, j_p, :]  for the 780 field pairs
(i, j), i < j, of F=40 fields (row-major triu order).

Architecture (8 NeuronCores, data-parallel over batch, B_loc=256):
  - "b-T" layout: stage-1 PE matmuls produce Y[(pair, e), b] in PSUM
    (pairs x e on partitions, batch on the free dim), so the final
    e-reduction runs on the PE as ones-mask matmuls (contraction over
    partitions), keeping the vector engine to a single multiply pass.
  - Tiles: one [128, 256] PSUM slice holds 2 pairs sharing one i-field:
    (i, 2t) and (i, 2t+1), matching xT chunk t (fields 2t / 2t+1 on the
    two partition halves). W is host-permuted (bf16) into per-tile
    contiguous lhsT blocks (zero blocks for invalid (i==2t, 2t) slots).
    4 tiles of one chunk share a 2-bank [128, 1024] PSUM group so one
    tensor_tensor covers 4 tiles (amortizes the DVE PSUM-access bubble).
  - Host pre-transposes x into the three layouts the kernel needs
    (xtc f32 / xtcb bf16 for the multiply, xtlo bf16 for stage-1 rhs),
    eliminating all on-device transposes.
  - stage 1: PE matmul Y = Wtile.T @ xT_i  (bf16, K=64, M=128, N=256).
  - stage 2: ACT evicts Y -> bf16 SBUF; DVE multiplies by xtcb chunk at
    the 2x packed rate -> z bf16 (leftover small groups multiply PSUM
    directly on DVE at 1x).
  - stage 3: PE ones-mask matmuls (K=128, M=32) accumulate 16 z-tiles
    into one PSUM bank = 32 output pair-rows (full fp32 accumulation).
    Col-group tiling is deliberately NOT used: tile_position col-groups
    interleaved with full-width matmuls corrupt nondeterministically on
    real TRN2 hardware (verified; CoreSim is clean).
  - ACT evicts each result bank; DMA to outT[bank*32 + row, b]; the
    host inverse-permutes pair rows and concatenates the batch shards.
"""

import numpy as np
import ml_dtypes

import concourse.bass as bass
import concourse.mybir as mybir
import concourse.tile as tile
from concourse import bacc
from concourse.bass_utils import run_bass_kernel_spmd

B, F, D = 2048, 40, 64
P = F * (F - 1) // 2  # 780
NCORES = 8
B_LOC = B // NCORES  # 256
F32 = mybir.dt.float32
BF16 = mybir.dt.bfloat16

NCHUNK = F // 2  # 20 xT chunks (2 fields each)
# tile list: (t, i) — pairs (i, 2t) [dummy if i==2t] and (i, 2t+1)
TILES = [(t, i) for t in range(NCHUNK) for i in range(2 * t + 1)]
NTILES = len(TILES)  # 400
REDUCE_COLTILE = False
TILES_PER_BANK = 64 if REDUCE_COLTILE else 16
BANK_ROWS = 128 if REDUCE_COLTILE else 32
NBANKS = (NTILES + TILES_PER_BANK - 1) // TILES_PER_BANK
OUT_ROWS = NBANKS * BANK_ROWS


GMAX = 4  # tiles per PSUM group (4 x 256 cols = 2 banks)
REDUCE_DELAY = 9  # groups of reduce-matmul lag (software pipelining)


def _build_groups():
    # pairs of same-chunk tiles sharing one [128,512] PSUM bank; split at
    # bank boundaries so both reduce slots land in the same bank-pass
    groups = []
    k = 0
    for t in range(NCHUNK):
        ilist = list(range(2 * t + 1))
        while ilist:
            take = min(GMAX, len(ilist), TILES_PER_BANK - (k % TILES_PER_BANK))
            groups.append((t, ilist[:take]))
            ilist = ilist[take:]
            k += take
    return groups


GROUPS = _build_groups()

WDMA_BATCH = 8  # stage-1 lhsT tiles per DMA


def host_prep(W: np.ndarray):
    """Build Wt3 [64, NTILES*128] f32, ONES [128, 512] bf16, PERM info."""
    # Wt2[d, p, e]
    Wt2 = np.ascontiguousarray(W.transpose(1, 0, 2))  # [64, 780, 64]
    pair_idx = -np.ones((F, F), dtype=np.int64)
    k = 0
    for i in range(F):
        for j in range(i + 1, F):
            pair_idx[i, j] = k
            k += 1
    Wt3 = np.zeros((D, NTILES * 128), dtype=np.float32)  # cast to bf16 at end
    # rows[k] = (origA or -1, origB) for tile k
    rows = []
    for k, (t, i) in enumerate(TILES):
        jA, jB = 2 * t, 2 * t + 1
        pA = pair_idx[i, jA] if i < jA else -1
        pB = pair_idx[i, jB]
        if pA >= 0:
            Wt3[:, k * 128 : k * 128 + 64] = Wt2[:, pA, :]
        Wt3[:, k * 128 + 64 : k * 128 + 128] = Wt2[:, pB, :]
        rows.append((pA, pB))
    # ones masks: ONES[:, q*32+m] — slot q (0..15): col 2q active for k<64,
    # col 2q+1 active for k>=64
    ones = np.zeros((128, 512), dtype=np.float32)
    for q in range(16):
        ones[0:64, q * 32 + 2 * q] = 1.0
        ones[64:128, q * 32 + 2 * q + 1] = 1.0
    ones = ones.astype(ml_dtypes.bfloat16)
    # out row of tile k: bank = k//64, s = k%64, g = s%4, q = s//4
    # rowA = bank*128 + 32*g + 2*q ; rowB = rowA + 1
    perm_src = np.zeros(P, dtype=np.int64)  # outT row for original pair p
    for k, (pA, pB) in enumerate(rows):
        bank, s = divmod(k, TILES_PER_BANK)
        if REDUCE_COLTILE:
            g, q = s % 4, s // 4
            rowA = bank * BANK_ROWS + 32 * g + 2 * q
        else:
            rowA = bank * BANK_ROWS + 2 * s
        if pA >= 0:
            perm_src[pA] = rowA
        perm_src[pB] = rowA + 1
    return Wt3.astype(ml_dtypes.bfloat16), ones, perm_src


def build_nc():
    nc = bacc.Bacc("TRN2", target_bir_lowering=False, debug=False)

    xtc_dram = nc.dram_tensor(
        "xtc", [128, NCHUNK * B_LOC], F32, kind="ExternalInput"
    ).ap()
    xtlo_dram = nc.dram_tensor(
        "xtlo", [64, F * B_LOC], BF16, kind="ExternalInput"
    ).ap()
    xtcb_dram = nc.dram_tensor(
        "xtcb", [128, NCHUNK * B_LOC], BF16, kind="ExternalInput"
    ).ap()
    wt_dram = nc.dram_tensor("Wt3", [D, NTILES * 128], BF16, kind="ExternalInput").ap()
    ones_dram = nc.dram_tensor("ones", [128, 512], BF16, kind="ExternalInput").ap()
    out_dram = nc.dram_tensor("outT", [OUT_ROWS, B_LOC], F32, kind="ExternalOutput").ap()

    with tile.TileContext(nc) as tc:
        with (
            tc.tile_pool(name="persist", bufs=1) as persist,
            tc.tile_pool(name="wpool", bufs=4) as wpool,
            tc.tile_pool(name="zpool", bufs=14) as zpool,
            tc.tile_pool(name="ybfpool", bufs=6) as ybfpool,
            tc.tile_pool(name="opool", bufs=2) as opool,
            tc.tile_pool(name="ypsum", bufs=3, space=bass.MemorySpace.PSUM) as ypsum,
            tc.tile_pool(name="rpsum", bufs=2, space=bass.MemorySpace.PSUM) as rpsum,
        ):
            ones = persist.tile([128, 512], BF16, tag="ones")
            nc.sync.dma_start(out=ones[:], in_=ones_dram[:])

            # XTC[(f%2)*64 + d, t*256 + m*128 + b]  (f = 2t + f%2) and the
            # low-half layout (all fields at partitions 0-63) are both
            # pre-transposed on the host and DMA'd directly.
            xtc = persist.tile([128, NCHUNK * B_LOC], F32, tag="xtc")
            xtlo = persist.tile([64, F * B_LOC], BF16, tag="xtlo")
            xtcb = persist.tile([128, NCHUNK * B_LOC], BF16, tag="xtcb")
            nq = NCHUNK * B_LOC // 4
            nf = F * B_LOC // 4
            for c4 in range(4):
                nc.sync.dma_start(
                    out=xtc[:, c4 * nq : (c4 + 1) * nq],
                    in_=xtc_dram[:, c4 * nq : (c4 + 1) * nq],
                )
                nc.sync.dma_start(
                    out=xtlo[:, c4 * nf : (c4 + 1) * nf],
                    in_=xtlo_dram[:, c4 * nf : (c4 + 1) * nf],
                )
                nc.sync.dma_start(
                    out=xtcb[:, c4 * nq : (c4 + 1) * nq],
                    in_=xtcb_dram[:, c4 * nq : (c4 + 1) * nq],
                )

            rbs = [None]
            wchunk = None
            k = 0
            nquad = 0
            pending = []

            def emit_reduce(z, k0, gsz):
                # accumulate into reduce bank via ones-mask matmuls
                for idx in range(gsz):
                    kt = k0 + idx
                    bank, s = divmod(kt, TILES_PER_BANK)
                    q = s
                    if s == 0:
                        rbs[0] = rpsum.tile([128, B_LOC], F32, tag="rb", name="rb")
                    rb = rbs[0]
                    last_in_bank = (s == TILES_PER_BANK - 1) or (kt == NTILES - 1)
                    nc.tensor.matmul(
                        rb[0:32, :],
                        ones[:, q * 32 : (q + 1) * 32],
                        z[:, idx * B_LOC : (idx + 1) * B_LOC],
                        start=(s == 0),
                        stop=last_in_bank,
                        tile_position=(0, 0),
                        skip_group_check=True,
                    )
                    if last_in_bank:
                        ob = opool.tile([BANK_ROWS, B_LOC], F32, tag="ob")
                        nc.vector.tensor_copy(out=ob[:], in_=rb[0:BANK_ROWS, :])
                        nc.sync.dma_start(
                            out=out_dram[
                                bank * BANK_ROWS : (bank + 1) * BANK_ROWS, :
                            ],
                            in_=ob[:],
                        )

            for t, ilist in GROUPS:
                gsz = len(ilist)
                # stage 1: Y[(p, e), b] = Wtile.T @ xT_i — gsz tiles share
                # one PSUM bank (disjoint column halves)
                y = ypsum.tile([128, GMAX * B_LOC], F32, tag="y")
                for idx, i in enumerate(ilist):
                    kt = k + idx
                    if kt % WDMA_BATCH == 0:
                        nw = min(WDMA_BATCH, NTILES - kt)
                        wchunk = wpool.tile([64, WDMA_BATCH * 128], BF16, tag="w")
                        nc.sync.dma_start(
                            out=wchunk[:, : nw * 128],
                            in_=wt_dram[:, kt * 128 : (kt + nw) * 128],
                        )
                    kk = kt % WDMA_BATCH
                    nc.tensor.matmul(
                        y[:, idx * B_LOC : (idx + 1) * B_LOC],
                        wchunk[:, kk * 128 : (kk + 1) * 128],
                        xtlo[:, i * B_LOC : (i + 1) * B_LOC],
                        start=True,
                        stop=True,
                    )

                # stage 2: z = Y * xT[j-fields chunk t]  (bf16 out), one TT
                # per group with stride-0 broadcast of the xtc chunk.
                # A fraction of pair-groups takes the ACT-evict + GPSIMD
                # multiply path to unload the DVE.
                z = zpool.tile([128, GMAX * B_LOC], BF16, tag="z")
                if gsz > 1:
                    nquad += 1
                    act_path = gsz == GMAX and (nquad % 4) < 2
                    if gsz == GMAX:
                        nquad += 1
                    if act_path:
                        # ACT evicts PSUM -> bf16, DVE multiplies at 2x
                        ybf = ybfpool.tile([128, GMAX * B_LOC], BF16, tag="ybf")
                        if nquad % 9 == 1:
                            # spill ~1/18 of the PSUM drains to the DVE to
                            # equalize ACT/DVE/PE busy time
                            nc.vector.tensor_copy(out=ybf[:], in_=y[:])
                        else:
                            nc.scalar.copy(out=ybf[:], in_=y[:])
                        in1 = xtcb[
                            :, None, t * B_LOC : (t + 1) * B_LOC
                        ].to_broadcast([128, gsz, B_LOC])
                        nc.vector.tensor_tensor(
                            z[:, : gsz * B_LOC].rearrange(
                                "p (n b) -> p n b", n=gsz
                            ),
                            ybf[:, : gsz * B_LOC].rearrange(
                                "p (n b) -> p n b", n=gsz
                            ),
                            in1,
                            mybir.AluOpType.mult,
                        )
                    else:
                        in1 = xtc[
                            :, None, t * B_LOC : (t + 1) * B_LOC
                        ].to_broadcast([128, gsz, B_LOC])
                        nc.vector.tensor_tensor(
                            z[:, : gsz * B_LOC].rearrange(
                                "p (n b) -> p n b", n=gsz
                            ),
                            y[:, : gsz * B_LOC].rearrange(
                                "p (n b) -> p n b", n=gsz
                            ),
                            in1,
                            mybir.AluOpType.mult,
                        )
                else:
                    nc.vector.tensor_tensor(
                        z[:, :B_LOC],
                        y[:, :B_LOC],
                        xtc[:, t * B_LOC : (t + 1) * B_LOC],
                        mybir.AluOpType.mult,
                    )

                # stage 3 is software-pipelined: queue this group's reduce
                # and emit the one from REDUCE_DELAY groups ago, so the PE's
                # in-order queue never waits on the just-issued ACT->DVE
                # multiply chain.
                pending.append((z, k, gsz))
                if len(pending) > REDUCE_DELAY:
                    emit_reduce(*pending.pop(0))
                k += gsz

            while pending:
                emit_reduce(*pending.pop(0))

    nc.compile()
    return nc


_NC = None


def kernel(x: np.ndarray, W: np.ndarray) -> np.ndarray:
    global _NC
    x = np.ascontiguousarray(np.asarray(x, dtype=np.float32))
    W = np.ascontiguousarray(np.asarray(W, dtype=np.float32))
    assert x.shape == (B, F, D) and W.shape == (P, D, D)

    Wt3, ones, perm_src = host_prep(W)

    if _NC is None:
        _NC = build_nc()

    in_maps = []
    for c in range(NCORES):
        xs = x[c * B_LOC : (c + 1) * B_LOC]  # [256, 40, 64]
        v = xs.transpose(1, 2, 0).reshape(NCHUNK, 2, D, B_LOC)
        xtc = np.ascontiguousarray(
            v.transpose(1, 2, 0, 3).reshape(128, NCHUNK * B_LOC)
        )
        xtlo = np.ascontiguousarray(
            xs.transpose(2, 1, 0).reshape(D, F * B_LOC)
        ).astype(ml_dtypes.bfloat16)
        xtcb = xtc.astype(ml_dtypes.bfloat16)
        in_maps.append(
            {"xtc": xtc, "xtcb": xtcb, "xtlo": xtlo, "Wt3": Wt3, "ones": ones}
        )
    res = run_bass_kernel_spmd(_NC, in_maps, core_ids=list(range(NCORES)))
    out = np.empty((B, P), dtype=np.float32)
    for c in range(NCORES):
        outT = res.results[c]["outT"]  # [OUT_ROWS, B_LOC]
        out[c * B_LOC : (c + 1) * B_LOC, :] = outT[perm_src, :].T
    return out
